# revision 11
# baseline (speedup 1.0000x reference)
"""Multi-head self-attention (RoPE, causal softmax) — Trainium2 Bass kernel.

Sharding over 8 NeuronCores: batch (2) x head-groups (16 heads / 4 groups).
Each core handles one batch element and 4 heads:
  - q/k/v projections for its heads (contraction over d=1024 on PE)
  - RoPE applied in a "halved" head-dim layout (even dims in rows 0..31,
    odd dims in rows 32..63 of each head block) so the rotation is a pure
    partition-aligned elementwise combine of two matmul outputs
  - causal attention: scoresT[m, n] blocks on PE, exp on ACT (scale=1/8
    fused), causal mask via gpsimd.affine_select, attn @ v with an extra
    ones-column in v so the softmax denominator falls out of the same
    matmul (row 64 of the output)
  - partial output projection with this core's slice of Wo columns
Host sums the 4 partial outputs per batch and transposes back.

All matmuls run in float32r (full-rate fp32 on the PE array).
"""

from contextlib import ExitStack

import numpy as np

import concourse.bass as bass
import concourse.bacc as bacc
import concourse.tile as tile
from concourse import mybir
from concourse.bass_utils import run_bass_kernel_spmd

# problem shape (hardcoded: graded standalone)
B, S, D, H, DK = 2, 2048, 1024, 16, 64
NCORES = 8
GROUPS = NCORES // B  # 4 head-groups (cores) per batch element
NH = H // GROUPS      # 4 heads per core
ROPE_THETA = 10000.0

P = 128
NQ = 512              # query-block (matmul moving free dim)
NT = S // NQ          # 4 query blocks
NM = S // P           # 16 key blocks
KT = D // P           # 8 contraction tiles for the x-projections
OT = NH * DK // P     # 2 stacked head-pair tiles for q/k

F32 = mybir.dt.float32
F32R = mybir.dt.float32r


def _r(ap):
    """Matmul inputs are already declared float32r."""
    return ap


def build_nc():
    nc = bacc.Bacc("TRN2", target_bir_lowering=False, debug=False)

    xt = nc.dram_tensor("xt", [D, S], F32R, kind="ExternalInput")        # x[b].T
    wq = nc.dram_tensor("wq", [D, NH * DK], F32R, kind="ExternalInput")  # perm'd, T
    wqr = nc.dram_tensor("wqr", [D, NH * DK], F32R, kind="ExternalInput")
    wk = nc.dram_tensor("wk", [D, NH * DK], F32R, kind="ExternalInput")
    wkr = nc.dram_tensor("wkr", [D, NH * DK], F32R, kind="ExternalInput")
    wv = nc.dram_tensor("wv", [D, NH * DK], F32R, kind="ExternalInput")
    wo = nc.dram_tensor("wo", [NH, DK, D], F32R, kind="ExternalInput")   # Wo cols, T
    cosr = nc.dram_tensor("cosr", [P, S], F32, kind="ExternalInput")
    sinr = nc.dram_tensor("sinr", [P, S], F32, kind="ExternalInput")    # sign-folded
    yt = nc.dram_tensor("yt", [D, S], F32, kind="ExternalOutput")       # partial y.T

    ones_dram = nc.dram_tensor("ones", [P, DK], F32R, kind="ExternalInput")

    with tile.TileContext(nc) as tc, ExitStack() as ctx, \
            nc.allow_low_precision(reason="float32r matmul inputs (~19-bit mantissa) are within tolerance"):
        # ---- persistent SBUF (whole kernel) ----
        persist = ctx.enter_context(tc.tile_pool(name="persist", bufs=1))
        cos_sb = persist.tile([P, S], F32, name="cos_sb")
        sin_sb = persist.tile([P, S], F32, name="sin_sb")
        ones_sb = persist.tile([P, DK], F32R, name="ones_sb")
        qtr_sb = persist.tile([P, OT, S], F32R, name="qtr_sb")    # rope(q)^T stacked pairs
        ktr_sb = persist.tile([P, OT, S], F32R, name="ktr_sb")
        vaug_sb = persist.tile([P, NM, NH, 65], F32R, name="vaug_sb")
        outt_sb = persist.tile([DK, NH * S], F32R, name="outt_sb")  # per-head out^T

        nc.gpsimd.dma_start(cos_sb[:], cosr[:, :])
        nc.gpsimd.dma_start(sin_sb[:], sinr[:, :])
        nc.sync.dma_start(ones_sb[:], ones_dram[:, :])
        # ones column of v_aug (col 64 of each head block)
        ones_src = ones_dram[:, :].rearrange("p (a b) -> p a b", a=NM)
        nc.sync.dma_start(vaug_sb[:, :, :, DK], ones_src)

        # ================= phase 1: projections + rope =================
        with (
            tc.tile_pool(name="xt_pool", bufs=1) as xt_pool,
            tc.tile_pool(name="w_pool", bufs=1) as w_pool,
            tc.tile_pool(name="proj_ps", bufs=4, space="PSUM") as proj_ps,
            tc.tile_pool(name="rope_tmp", bufs=3) as rope_tmp,
        ):
            xt_sb = xt_pool.tile([P, KT, S], F32R, name="xt_sb")
            for k in range(KT):
                nc.sync.dma_start(xt_sb[:, k, :], xt[k * P:(k + 1) * P, :])

            w_sbs = {}
            for name, drt in (("wq", wq), ("wqr", wqr), ("wk", wk),
                              ("wkr", wkr), ("wv", wv)):
                t = w_pool.tile([P, KT, NH * DK], F32R, name=f"{name}_sb")
                for k in range(KT):
                    nc.sync.dma_start(t[:, k, :], drt[k * P:(k + 1) * P, :])
                w_sbs[name] = t

            # q/k projections with rope, head pairs stacked on partitions
            for dst, wa, wb in ((qtr_sb, "wq", "wqr"), (ktr_sb, "wk", "wkr")):
                for t in range(OT):
                    for j in range(NT):
                        ps = proj_ps.tile([P, NQ], F32, name="proj_main", tag="proj")
                        psr = proj_ps.tile([P, NQ], F32, name="proj_rot", tag="proj")
                        for k in range(KT):
                            nc.tensor.matmul(
                                ps[:],
                                lhsT=_r(w_sbs[wa][:, k, t * P:(t + 1) * P]),
                                rhs=_r(xt_sb[:, k, j * NQ:(j + 1) * NQ]),
                                start=(k == 0), stop=(k == KT - 1))
                        for k in range(KT):
                            nc.tensor.matmul(
                                psr[:],
                                lhsT=_r(w_sbs[wb][:, k, t * P:(t + 1) * P]),
                                rhs=_r(xt_sb[:, k, j * NQ:(j + 1) * NQ]),
                                start=(k == 0), stop=(k == KT - 1))
                        t1 = rope_tmp.tile([P, NQ], F32, name="rope_a", tag="rope")
                        t2 = rope_tmp.tile([P, NQ], F32, name="rope_b", tag="rope")
                        csl = slice(j * NQ, (j + 1) * NQ)
                        nc.vector.tensor_mul(t1[:], ps[:], cos_sb[:, csl])
                        nc.vector.tensor_mul(t2[:], psr[:], sin_sb[:, csl])
                        nc.vector.tensor_add(dst[:, t, csl], t1[:], t2[:])

            # v projection -> v_aug[:, st, h*65 : h*65+64]
            for st in range(NM):
                psv = proj_ps.tile([P, NH * DK], F32, name="proj_v", tag="proj")
                for k in range(KT):
                    nc.tensor.matmul(
                        psv[:],
                        lhsT=_r(xt_sb[:, k, st * P:(st + 1) * P]),
                        rhs=_r(w_sbs["wv"][:, k, :]),
                        start=(k == 0), stop=(k == KT - 1))
                dst = vaug_sb[:, st, :, 0:DK]
                src = psv[:].rearrange("p (h c) -> p h c", h=NH)
                nc.vector.tensor_copy(dst, src)

        # ================= phase 2: attention =================
        with (
            tc.tile_pool(name="score_ps", bufs=3, space="PSUM") as score_ps,
            tc.tile_pool(name="oaug_ps", bufs=2, space="PSUM") as oaug_ps,
            tc.tile_pool(name="bcast_ps", bufs=1, space="PSUM") as bcast_ps,
            tc.tile_pool(name="exp_pool", bufs=8) as exp_pool,
            tc.tile_pool(name="recip_pool", bufs=4) as recip_pool,
            tc.tile_pool(name="bcast_sb", bufs=4) as bcast_sb_pool,
        ):
            for pr in range(NH // 2):          # head pairs
                heads = (2 * pr, 2 * pr + 1)
                for j in range(NT):
                    imax = 4 * j + 3
                    oa = {}
                    for h in heads:
                        oa[h] = oaug_ps.tile([DK + 1, NQ], F32, name="oaug")
                    for i in range(imax + 1):
                        for h in heads:
                            hb = (h % 2) * DK
                            sc = score_ps.tile([P, NQ], F32, name="score")
                            nc.tensor.matmul(
                                sc[:],
                                lhsT=_r(ktr_sb[hb:hb + DK, h // 2,
                                               i * P:(i + 1) * P]),
                                rhs=_r(qtr_sb[hb:hb + DK, h // 2,
                                              j * NQ:(j + 1) * NQ]),
                                start=True, stop=True)
                            eb = exp_pool.tile([P, NQ], F32R, name="expblk")
                            nc.scalar.activation(
                                eb[:], sc[:], mybir.ActivationFunctionType.Exp,
                                scale=float(1.0 / np.sqrt(DK)))
                            if i >= 4 * j:   # diagonal block: causal mask
                                nc.gpsimd.affine_select(
                                    out=eb[:], in_=eb[:],
                                    compare_op=mybir.AluOpType.is_ge,
                                    fill=0.0,
                                    base=-(P * (i - 4 * j)),
                                    channel_multiplier=-1,
                                    pattern=[[1, NQ]])
                            nc.tensor.matmul(
                                oa[h][:],
                                lhsT=_r(vaug_sb[:, i, h, :]),
                                rhs=_r(eb[:]),
                                start=(i == 0), stop=(i == imax))
                    for h in heads:
                        # softmax denominator is row 64 (the ones column of v)
                        rc = recip_pool.tile([DK + 1, NQ], F32R, name="recip")
                        nc.vector.reciprocal(rc[DK:DK + 1, :], oa[h][DK:DK + 1, :])
                        bc = bcast_ps.tile([DK, NQ], F32, name="bcastp")
                        nc.tensor.matmul(
                            bc[:],
                            lhsT=_r(ones_sb[DK:DK + 1, :]),
                            rhs=_r(rc[DK:DK + 1, :]),
                            start=True, stop=True)
                        bs = bcast_sb_pool.tile([DK, NQ], F32, name="bcast_s")
                        nc.vector.tensor_copy(bs[:], bc[:])
                        nc.vector.tensor_mul(
                            outt_sb[:, h * S + j * NQ: h * S + (j + 1) * NQ],
                            oa[h][0:DK, :], bs[:])

        # ================= phase 3: output projection =================
        with (
            tc.tile_pool(name="wo_pool", bufs=1) as wo_pool,
            tc.tile_pool(name="wo_ps", bufs=4, space="PSUM") as wo_ps,
            tc.tile_pool(name="fin_pool", bufs=4) as fin_pool,
        ):
            wo_sb = wo_pool.tile([DK, NH, D], F32R, name="wo_sb")
            for h in range(NH):
                nc.sync.dma_start(wo_sb[:, h, :], wo[h, :, :])
            for ot in range(D // P):
                for j in range(NT):
                    ps = wo_ps.tile([P, NQ], F32, name="wops")
                    for h in range(NH):
                        nc.tensor.matmul(
                            ps[:],
                            lhsT=_r(wo_sb[:, h, ot * P:(ot + 1) * P]),
                            rhs=_r(outt_sb[:, h * S + j * NQ: h * S + (j + 1) * NQ]),
                            start=(h == 0), stop=(h == NH - 1))
                    fin = fin_pool.tile([P, NQ], F32, name="fin")
                    nc.vector.tensor_copy(fin[:], ps[:])
                    nc.sync.dma_start(
                        yt[ot * P:(ot + 1) * P, j * NQ:(j + 1) * NQ], fin[:])

    nc.compile()
    return nc


_NC_CACHE = {}


def _get_nc():
    if "nc" not in _NC_CACHE:
        _NC_CACHE["nc"] = build_nc()
    return _NC_CACHE["nc"]


_HALF = DK // 2
_PERM = np.concatenate([np.arange(0, DK, 2), np.arange(1, DK, 2)])


def _prep_qk(W, heads):
    """Per-head RoPE-permuted projection weights and their half-swapped twin."""
    Wh = W.reshape(H, DK, D)[heads][:, _PERM, :]                    # [NH, DK, D]
    Wrot = np.concatenate([Wh[:, _HALF:, :], Wh[:, :_HALF, :]], axis=1)
    return (np.ascontiguousarray(Wh.reshape(NH * DK, D).T),
            np.ascontiguousarray(Wrot.reshape(NH * DK, D).T))


def make_in_maps(x, token_positions, Wq, Wk, Wv, Wo):
    x = np.asarray(x, dtype=np.float32)
    Wq = np.asarray(Wq, dtype=np.float32)
    Wk = np.asarray(Wk, dtype=np.float32)
    Wv = np.asarray(Wv, dtype=np.float32)
    Wo = np.asarray(Wo, dtype=np.float32)
    pos = np.asarray(token_positions)

    j = np.arange(_HALF, dtype=np.float64)
    inv_freq = ROPE_THETA ** (-2.0 * j / DK)                        # [32]

    in_maps = []
    for core in range(NCORES):
        b = core // GROUPS
        g = core % GROUPS
        heads = list(range(g * NH, (g + 1) * NH))
        wq_, wqr_ = _prep_qk(Wq, heads)
        wk_, wkr_ = _prep_qk(Wk, heads)
        wv_ = np.ascontiguousarray(Wv.reshape(H, DK, D)[heads].reshape(NH * DK, D).T)
        wo_ = np.ascontiguousarray(Wo.T.reshape(H, DK, D)[heads])
        ang = np.outer(inv_freq, pos[b].astype(np.float64))          # [32, S]
        cos32 = np.cos(ang)
        sin32 = np.sin(ang)
        cosr = np.tile(cos32, (4, 1)).astype(np.float32)             # [128, S]
        sinr = np.tile(np.concatenate([-sin32, sin32], axis=0),
                       (2, 1)).astype(np.float32)
        in_maps.append({
            "xt": np.ascontiguousarray(x[b].T),
            "wq": wq_, "wqr": wqr_, "wk": wk_, "wkr": wkr_,
            "wv": wv_, "wo": wo_, "cosr": cosr, "sinr": sinr,
            "ones": np.ones((P, DK), dtype=np.float32),
        })
    return in_maps


def _gather(results):
    outs = [np.asarray(r["yt"], dtype=np.float32) for r in results]
    y = np.stack([
        sum(outs[b * GROUPS + 1: (b + 1) * GROUPS], outs[b * GROUPS]).T
        for b in range(B)
    ])
    return np.ascontiguousarray(y)


def kernel(x, token_positions, Wq, Wk, Wv, Wo):
    in_maps = make_in_maps(x, token_positions, Wq, Wk, Wv, Wo)
    res = run_bass_kernel_spmd(_get_nc(), in_maps, core_ids=list(range(NCORES)))
    return _gather(res.results)


def kernel_traced(x, token_positions, Wq, Wk, Wv, Wo, **kwargs):
    """Like kernel() but with NTFF tracing; returns (output, BassKernelResults)."""
    in_maps = make_in_maps(x, token_positions, Wq, Wk, Wv, Wo)
    res = run_bass_kernel_spmd(_get_nc(), in_maps, core_ids=list(range(NCORES)),
                               trace=True, **kwargs)
    return _gather(res.results), res
